# revision 28
# baseline (speedup 1.0000x reference)
"""Trainium2 (8 NeuronCores) kernel for a dense causal multi-head attention block.

Problem shapes: B=2, S=2048, D=2048, H=16, DH=128 (fp32 in/out).

Distribution (sharding_hint: tensor-parallel over heads): core c owns heads
{2c, 2c+1}. v2 restructure of the baseline, targeting full PE occupancy:

  Projections (b-outer, streamed): for each batch, X^T arrives as 4 s-chunks
    of [128, 16, 512] (double-buffered, first chunk split across many DMA
    rings so the PE starts ~8us in). Per chunk, 6 matmul chains (2 heads x
    Q/K/V) of 16 accumulate in PSUM; ACT evicts with the bias into
    QT/KT/VT[hl][b] ([dh, s] layout).
  Attention (hl-outer for collective overlap): per (hl, b): V_kd via PE
    transposes (4 per PSUM bank, single eviction), then per qc chunk a
    4-deep scores pipeline:
       scores^T = K_tile.T @ Q_chunk (PE) -> exp on ACT (from PSUM) ->
       diagonal-strip mask (GPSIMD mult) -> bf16 partial-sum chains (DVE) ->
       z^T += V_tile.T @ pexp (PE, PSUM accum).
    Softmax denominator off the PE: chain merges (DVE) ->
    partition_all_reduce (GPSIMD, broadcast sum) -> reciprocal_approx_fast
    (custom DVE op, fp32) -> zs = z^T * rden (DVE) -> DMA to a2a slot.
  AllToAll per hl (2 MB bf16): hl=0's hides under hl=1 attention; hl=1's
    under the even-head half of the output projection.
  Phase 2 (row-parallel): out[q, d] = sum_h Z^T_h.T @ W_O_h + b_O for this
    core's 512 rows; even-head half accumulates into an fp32 stash (with
    b_O folded in), odd-head half lands after the second AllToAll; W_O and
    the Z tiles are prefetched on otherwise-idle DMA queues.
"""

import numpy as np
import ml_dtypes

import concourse.bass as bass
import concourse.bass_isa as bass_isa
import concourse.mybir as mybir
import concourse.tile as tile
from concourse import bacc
from concourse.bass import ts
from concourse.bass_utils import run_bass_kernel_spmd
from concourse.masks import make_identity

B, S, D, H, DH = 2, 2048, 2048, 16, 128
NCORES = 8
HL = H // NCORES            # heads per core = 2
QB = (B * S) // NCORES      # output rows per core = 512
P = 128
SC = 512                    # free-dim chunk (PSUM bank = 512 fp32)
NSC = S // SC               # 4
NDT = D // P                # 16 contraction tiles for D
NST = S // P                # 16 sequence tiles of 128
NQT = QB // P               # 4 local q tiles in phase 2
NDC = D // SC               # 4 output-dim chunks
SCALE = 1.0 / float(np.sqrt(DH))

F32 = mybir.dt.float32
BF16 = mybir.dt.bfloat16


def build_nc():
    nc = bacc.Bacc("TRN2", target_bir_lowering=False, debug=False,
                   num_devices=NCORES)

    # xt = X^T per batch ([B, D, S]); weights pre-tiled partition-major on the
    # host so every DMA below is contiguous.
    xt = nc.dram_tensor("xt", [B, NSC, NDT, P, SC], BF16,
                        kind="ExternalInput")
    wq = nc.dram_tensor("wq", [HL, P, NDT, DH], BF16, kind="ExternalInput")
    wk = nc.dram_tensor("wk", [HL, P, NDT, DH], BF16, kind="ExternalInput")
    wv = nc.dram_tensor("wv", [HL, P, NDT, DH], BF16, kind="ExternalInput")
    bq = nc.dram_tensor("bq", [DH, HL], F32, kind="ExternalInput")
    bk = nc.dram_tensor("bk", [DH, HL], F32, kind="ExternalInput")
    bv = nc.dram_tensor("bv", [DH, HL], F32, kind="ExternalInput")
    # wo[p, h, d] = W_O[h, p, d]
    wo = nc.dram_tensor("wo", [H, P, D], BF16, kind="ExternalInput")
    bo = nc.dram_tensor("bo", [1, D], BF16, kind="ExternalInput")
    out = nc.dram_tensor("out", [QB, D], F32, kind="ExternalOutput")

    Exp = mybir.ActivationFunctionType.Exp
    Ident = mybir.ActivationFunctionType.Identity
    AddOp = bass_isa.ReduceOp.add

    with tile.TileContext(nc) as tc:
        with (
            tc.tile_pool(name="const", bufs=1) as cpool,
            tc.tile_pool(name="dram", bufs=1, space="DRAM") as dpool,
            tc.tile_pool(name="wo_p", bufs=1) as wopool,
            tc.tile_pool(name="qkt", bufs=1) as qktpool,
            tc.tile_pool(name="vt", bufs=2) as vtpool,
            tc.tile_pool(name="small", bufs=4) as spool,
            tc.tile_pool(name="ps_acc", bufs=5, space="PSUM") as ps_acc,
            tc.tile_pool(name="ps_den", bufs=1, space="PSUM") as ps_den,
            tc.tile_pool(name="ps_z", bufs=2, space="PSUM") as ps_z,
        ):
            # QT/KT per (hl, b) persist through attention; VT dies into V_kd.
            QT = {}
            KT = {}
            for hl in range(HL):
                for b in range(B):
                    QT[hl, b] = qktpool.tile([P, S], BF16, tag=f"qt{hl}{b}",
                                             name=f"qt{hl}{b}")
                    KT[hl, b] = qktpool.tile([P, S], BF16, tag=f"kt{hl}{b}",
                                             name=f"kt{hl}{b}")

            # one AllToAll per local head index
            a2a_in = [dpool.tile([NCORES, P, SC], BF16, tag=f"a2a_in{hl}",
                                 name=f"a2a_in{hl}") for hl in range(HL)]
            a2a_out = [dpool.tile([NCORES, P, SC], BF16, tag=f"a2a_out{hl}",
                                  name=f"a2a_out{hl}") for hl in range(HL)]

            def emit_attn(hl, b):
                    V_kd = VKD[hl, b]
                    for qc in range(NSC):
                        z_ps = ps_z.tile([P, SC], F32, tag="z")
                        nkt = 4 * qc + 4
                        nch = min(4, nkt)
                        pexps = {}
                        chains = {}

                        def emit_scores(kt, qc=qc, hl=hl, b=b, chains=chains,
                                        nch=nch, pexps=None):
                            j = kt - 4 * qc
                            lo = 128 * j if j >= 0 else 0
                            s_ps = ps_acc.tile([P, SC], F32, tag="acc")
                            nc.tensor.matmul(
                                s_ps[:, :SC - lo],
                                lhsT=KT[hl, b][:, ts(kt, P)],
                                rhs=QT[hl, b][:, qc * SC + lo:(qc + 1) * SC],
                                start=True, stop=True)
                            pexp = spool.tile([P, SC], BF16, tag="p", bufs=10)
                            nc.scalar.activation(
                                pexp[:, lo:], s_ps[:, :SC - lo], Exp,
                                bias=0.0, scale=SCALE)
                            if j >= 0:
                                nc.vector.tensor_mul(
                                    pexp[:, lo:lo + P], pexp[:, lo:lo + P],
                                    tri)
                            c = kt % nch
                            if c not in chains:
                                chains[c] = pexp
                            else:
                                nc.vector.tensor_add(
                                    chains[c][:, lo:], chains[c][:, lo:],
                                    pexp[:, lo:])
                            pexps[kt] = (pexp, lo)

                        def emit_den(qc=qc, chains=chains):
                            # pairwise merge -> ones-matmul (partition sum +
                            # broadcast in PSUM) -> fast fp32 reciprocal.
                            clo = [128 * c if qc == 0 else 0 for c in range(4)]
                            nc.vector.tensor_add(
                                chains[0][:, clo[1]:], chains[0][:, clo[1]:],
                                chains[1][:, clo[1]:])
                            nc.vector.tensor_add(
                                chains[2][:, clo[3]:], chains[2][:, clo[3]:],
                                chains[3][:, clo[3]:])
                            nc.vector.tensor_add(
                                chains[0][:, clo[2]:], chains[0][:, clo[2]:],
                                chains[2][:, clo[2]:])
                            rb_ps = ps_den.tile([P, SC], F32, tag="rb")
                            nc.tensor.matmul(rb_ps, lhsT=ones128,
                                             rhs=chains[0], start=True,
                                             stop=True)
                            rden = spool.tile([P, SC], F32, tag="rden",
                                              bufs=2)
                            nc.vector.reciprocal_approx_fast(rden, rb_ps)
                            return rden

                        for k0 in range(min(4, nkt)):
                            emit_scores(k0, pexps=pexps)
                        rden = None
                        for kt in range(nkt):
                            # z before the kt+4 prefetch: chain accumulation
                            # mutates the seed pexp tiles, so their z matmuls
                            # must be emitted first.
                            pexp, lo = pexps.pop(kt)
                            nc.tensor.matmul(
                                z_ps[:, lo:], lhsT=V_kd[:, kt, :],
                                rhs=pexp[:, lo:],
                                start=(kt == 0), stop=(kt == nkt - 1),
                                skip_group_check=True)
                            if kt + 4 < nkt:
                                emit_scores(kt + 4, pexps=pexps)
                                if kt + 4 == nkt - 1:
                                    rden = emit_den()
                        if nkt <= 4:
                            rden = emit_den()
                        zs = spool.tile([P, SC], BF16, tag="zs", bufs=2)
                        nc.vector.tensor_mul(zs, z_ps, rden)
                        for q4 in range(4):
                            nc.sync.dma_start(
                                a2a_in[hl][4 * b + qc][32 * q4:32 * q4 + 32],
                                zs[32 * q4:32 * q4 + 32, :])


            VT = {}
            VKD = {}
            with (
                tc.tile_pool(name="xt", bufs=6) as xtpool,
                tc.tile_pool(name="wpool", bufs=1) as wpool,
            ):
                # ---- head: kick the big streams first ----
                # First s-chunk of batch 0 split across many DMA rings.
                HD = NDT // 2
                xh0a = xtpool.tile([P, HD, SC], BF16, tag="xch", name="xh0a",
                                   bufs=6)
                xh0b = xtpool.tile([P, HD, SC], BF16, tag="xch", name="xh0b",
                                   bufs=6)
                for g in range(NDT // 2):
                    h = (xh0a, xh0b)[(2 * g) // HD]
                    nc.sync.dma_start(
                        h[:, (2 * g) % HD:(2 * g) % HD + 2, :],
                        xt.ap()[0][0][2 * g:2 * g + 2].transpose([1, 0, 2]))

                # weight tiles [d_part, d_tile, dh] per (proj, head)
                w_sb = []
                for hl in range(HL):
                    per = []
                    for nm, w in (("wq", wq), ("wk", wk), ("wv", wv)):
                        t_sb = wpool.tile([P, NDT, DH], BF16, tag=f"{nm}{hl}",
                                          name=f"{nm}{hl}")
                        for g in range(4):
                            nc.scalar.dma_start(
                                t_sb[:, 4 * g:4 * g + 4, :],
                                w.ap()[hl][:, 4 * g:4 * g + 4, :])
                        per.append(t_sb)
                    w_sb.append(per)

                # ---- constants ----
                ident = cpool.tile([P, P], BF16)
                make_identity(nc, ident)
                # tri[k, j] = 1.0 iff j >= k (upper triangle, diagonal tiles)
                tri = cpool.tile([P, P], BF16)
                nc.gpsimd.memset(tri, 1.0)
                nc.gpsimd.affine_select(
                    out=tri, in_=tri, compare_op=mybir.AluOpType.is_ge,
                    fill=0.0, base=0, pattern=[[1, P]], channel_multiplier=-1,
                )
                ones128 = cpool.tile([P, P], BF16, tag="ones128")
                nc.vector.memset(ones128, 1.0)
                ones_1p = cpool.tile([1, P], BF16, tag="ones_1p")
                nc.vector.memset(ones_1p, 1.0)
                bias_sb = {}
                for nm, t in (("q", bq), ("k", bk), ("v", bv)):
                    bb = cpool.tile([P, HL], F32, tag=f"b{nm}")
                    nc.sync.dma_start(bb, t.ap())
                    bias_sb[nm] = bb
                bo_sb = cpool.tile([1, D], BF16, tag="bo_sb")
                nc.scalar.dma_start(bo_sb, bo.ap())
                # warm the exp table while DMAs stream
                warm = cpool.tile([P, 1], BF16)
                warm2 = cpool.tile([P, 1], BF16)
                nc.vector.memset(warm, 0.0)
                nc.scalar.activation(warm2, warm, Exp, bias=0.0, scale=1.0)

                # ---- projections: b-outer, streamed s-chunks ----
                def emit_proj(b):
                    for sc in range(NSC):
                        if b == 0 and sc == 0:
                            xha, xhb = xh0a, xh0b
                        else:
                            xha = xtpool.tile([P, HD, SC], BF16, tag="xch",
                                              name=f"xh{b}{sc}a", bufs=6)
                            xhb = xtpool.tile([P, HD, SC], BF16, tag="xch",
                                              name=f"xh{b}{sc}b", bufs=6)
                            for g in range(NDT // 2):
                                h = (xha, xhb)[(2 * g) // HD]
                                nc.sync.dma_start(
                                    h[:, (2 * g) % HD:(2 * g) % HD + 2, :],
                                    xt.ap()[b][sc][2 * g:2 * g + 2]
                                    .transpose([1, 0, 2]))
                        for hl in range(HL):
                            if sc == 0:
                                VT[hl, b] = vtpool.tile(
                                    [P, S], BF16, tag="vt", name=f"vt{hl}{b}")
                            for pi, (dst, bcol) in enumerate((
                                (QT[hl, b], bias_sb["q"]),
                                (KT[hl, b], bias_sb["k"]),
                                (VT[hl, b], bias_sb["v"]),
                            )):
                                ps = ps_acc.tile([P, SC], F32, tag="acc")
                                wt = w_sb[hl][pi]
                                for dt_ in range(NDT):
                                    h = (xha, xhb)[dt_ // HD]
                                    nc.tensor.matmul(
                                        ps, lhsT=wt[:, dt_, :],
                                        rhs=h[:, dt_ % HD, :],
                                        start=(dt_ == 0),
                                        stop=(dt_ == NDT - 1))
                                nc.scalar.activation(
                                    dst[:, ts(sc, SC)], ps, Ident,
                                    bias=bcol[:, hl:hl + 1], scale=1.0)
                    # V in [k, dh] via PE transposes as soon as VT completes,
                    # releasing the VT buffers for the next batch.
                    if sc == NSC - 1:
                        for hl in range(HL):
                            vkd = vtpool.tile([P, NST, DH], BF16, tag="vkd",
                                              bufs=4, name=f"vkd{hl}{b}")
                            for st4 in range(NST // 4):
                                pst = ps_acc.tile([P, 4 * P], BF16, tag="acc")
                                for j in range(4):
                                    nc.tensor.transpose(
                                        pst[:, ts(j, P)],
                                        VT[hl, b][:, ts(4 * st4 + j, P)],
                                        ident)
                                nc.vector.tensor_copy(
                                    vkd[:, 4 * st4:4 * st4 + 4, :], pst)
                            VKD[hl, b] = vkd

                emit_proj(0)
                # W_O even half loads once the early xt stream has drained
                # its burst; it is only needed at the output projection.
                WO_ev = wopool.tile([P, H // 2, D], BF16, tag="woev")
                for j in range(H // 2):
                    nc.scalar.dma_start(WO_ev[:, j, :], wo.ap()[2 * j])
                emit_attn(0, 0)
                emit_proj(1)

            # ---- attention: hl-outer so a2a[0] hides under hl=1 ----
            p2cm = tc.tile_pool(name="p2", bufs=1)
            p2pool = p2cm.__enter__()
            # odd-head W_O half: descriptor generation happens now (gpsimd is
            # idle), transfers start as soon as the projection space frees.
            WO_od = p2pool.tile([P, H // 2, D], BF16, tag="wood")
            for j in range(H // 2):
                nc.gpsimd.dma_start(WO_od[:, j, :], wo.ap()[2 * j + 1])
            # hl=0's remaining window fires the first AllToAll with all of
            # hl=1's attention still queued: core-local work absorbs the
            # peer-arrival skew of the collective.
            emit_attn(0, 1)
            nc.gpsimd.collective_compute(
                "AllToAll", mybir.AluOpType.bypass,
                replica_groups=[list(range(NCORES))],
                ins=[a2a_in[0][:]], outs=[a2a_out[0][:]],
            )
            ZT_ev = p2pool.tile([P, NCORES, SC], BF16, tag="ztev")
            for j in range(NCORES):
                nc.gpsimd.dma_start(ZT_ev[:, j, :], a2a_out[0][j])
            emit_attn(1, 0)
            emit_attn(1, 1)
            nc.gpsimd.collective_compute(
                "AllToAll", mybir.AluOpType.bypass,
                replica_groups=[list(range(NCORES))],
                ins=[a2a_in[1][:]], outs=[a2a_out[1][:]],
            )

            # ---- phase 2: output projection for this core's 512 rows ----
            if True:
                bo_b = p2pool.tile([P, D], BF16, tag="bo_b")
                for dc in range(NDC):
                    bops = ps_acc.tile([P, SC], F32, tag="acc")
                    nc.tensor.matmul(bops, lhsT=ones_1p,
                                     rhs=bo_sb[:, ts(dc, SC)],
                                     start=True, stop=True)
                    nc.scalar.activation(bo_b[:, ts(dc, SC)], bops,
                                         Ident, bias=0.0, scale=1.0)
                part = {}
                for qt in range(NQT):
                    for dc in range(NDC):
                        ops = ps_acc.tile([P, SC], F32, tag="acc")
                        for j in range(NCORES):
                            nc.tensor.matmul(
                                ops, lhsT=ZT_ev[:, j, ts(qt, P)],
                                rhs=WO_ev[:, j, ts(dc, SC)],
                                start=(j == 0), stop=(j == NCORES - 1))
                        pt = p2pool.tile([P, SC], BF16, tag=f"part{qt}_{dc}",
                                         name=f"part{qt}_{dc}")
                        nc.vector.tensor_add(pt, ops, bo_b[:, ts(dc, SC)])
                        part[qt, dc] = pt
                ZT_od = p2pool.tile([P, NCORES, SC], BF16, tag="ztod")
                for j in range(NCORES):
                    nc.gpsimd.dma_start(ZT_od[:, j, :], a2a_out[1][j])
                for qt in range(NQT):
                    for dc in range(NDC):
                        ops = ps_acc.tile([P, SC], F32, tag="acc")
                        for j in range(NCORES):
                            nc.tensor.matmul(
                                ops, lhsT=ZT_od[:, j, ts(qt, P)],
                                rhs=WO_od[:, j, ts(dc, SC)],
                                start=(j == 0), stop=(j == NCORES - 1))
                        osb = p2pool.tile([P, SC], F32, tag="osb", bufs=2)
                        nc.vector.tensor_add(osb, ops, part[qt, dc])
                        for q4 in range(4):
                            nc.sync.dma_start(
                                out.ap()[qt * P + 32 * q4:qt * P + 32 * q4
                                         + 32, ts(dc, SC)],
                                osb[32 * q4:32 * q4 + 32, :])
            p2cm.__exit__(None, None, None)

    nc.compile()
    return nc


_CACHE = {}


def _get_nc():
    if "nc" not in _CACHE:
        _CACHE["nc"] = build_nc()
    return _CACHE["nc"]


def make_in_maps(resid_pre, W_Q, W_K, W_V, W_O, b_Q, b_K, b_V, b_O):
    bf = ml_dtypes.bfloat16
    x_bf = np.asarray(resid_pre, np.float32).astype(bf)
    xt_full = x_bf.transpose(0, 2, 1)  # [B, D, S]
    # pretile to [B, NSC, NDT, P, SC] so every DMA block is contiguous
    xt = np.ascontiguousarray(
        xt_full.reshape(B, NDT, P, NSC, SC).transpose(0, 3, 1, 2, 4))
    # weights pre-tiled to [H, P, NDT, DH]: w_t[h, p, o, k] = W[h, o*P + p, k]
    def tile_w(W):
        Wb = np.asarray(W, np.float32).astype(bf)
        return np.ascontiguousarray(
            Wb.reshape(H, NDT, P, DH).transpose(0, 2, 1, 3))
    WQ, WK, WV = tile_w(W_Q), tile_w(W_K), tile_w(W_V)
    # wo[h, p, d] = W_O[h, p, d] (contiguous per-head [P, D] blocks)
    WOf = np.ascontiguousarray(np.asarray(W_O, np.float32)).astype(bf)
    bQ = np.ascontiguousarray(np.asarray(b_Q, np.float32).T)  # [DH, H]
    bK = np.ascontiguousarray(np.asarray(b_K, np.float32).T)
    bV = np.ascontiguousarray(np.asarray(b_V, np.float32).T)
    bO = np.ascontiguousarray(
        np.asarray(b_O, np.float32)).reshape(1, D).astype(bf)
    in_maps = []
    for c in range(NCORES):
        hs = slice(c * HL, (c + 1) * HL)
        in_maps.append({
            "xt": xt,
            "wq": np.ascontiguousarray(WQ[hs]),
            "wk": np.ascontiguousarray(WK[hs]),
            "wv": np.ascontiguousarray(WV[hs]),
            "bq": np.ascontiguousarray(bQ[:, hs]),
            "bk": np.ascontiguousarray(bK[:, hs]),
            "bv": np.ascontiguousarray(bV[:, hs]),
            "wo": WOf,
            "bo": bO,
        })
    return in_maps


def assemble(results):
    out = np.empty((B, S, D), np.float32)
    for c in range(NCORES):
        b, r = divmod(c, NCORES // B)  # divmod(c, 4)
        out[b, r * QB:(r + 1) * QB] = results[c]["out"]
    return out


def kernel(resid_pre, W_Q, W_K, W_V, W_O, b_Q, b_K, b_V, b_O,
           _trace=False, _return_raw=False):
    nc = _get_nc()
    in_maps = make_in_maps(resid_pre, W_Q, W_K, W_V, W_O, b_Q, b_K, b_V, b_O)
    res = run_bass_kernel_spmd(nc, in_maps, core_ids=list(range(NCORES)),
                               trace=_trace)
    out = assemble(res.results)
    if _return_raw:
        return out, res
    return out
